# revision 27
# baseline (speedup 1.0000x reference)
"""Local-window multi-head attention (window=33) for Trainium2, 8-core SPMD.

Sharding: data-parallel over batch (B=8 -> 1 batch per core), weights
replicated. Per core, one fused Bass/Tile kernel:

  Phase 1: q/k projections over the full sequence into SBUF-resident
    qZ [128, H, S] fp16 (per-head, other co-projected head's partition
    half zeroed) and kT [128, 4, 16+S+112] fp16 (zero-padded halo).
  Phase 2: per 96-query block, software-pipelined 2 deep:
    v-projection (natural [kpos, dout]), pair-packed transposed score
    matmuls (N=192), exp with per-partition key penalty bias, band mask,
    MM2 + piggybacked denominator matmuls (shared stationary), softmax
    normalize, PE transpose to atT [din, q]; per 6-block chunk a
    transposed output projection oT[dout, q] with bias folded into the
    PSUM->SBUF copy. Output written transposed [D, S] fp16; host
    transposes back.

Dtypes: fp16 for x/weights/q/k/att (precision), bf16 for probabilities
and v (exp range), fp32 accumulation everywhere.
"""
import contextlib
import os
import sys
sys.path.insert(0, "/opt/trn_rl_repo")
import numpy as np

B, S, D, H, HD = 8, 4096, 512, 8, 64
WIN, HALF = 33, 16
QB = 96
NB = (S + QB - 1) // QB          # 43 blocks (42 full + one 64-wide)
CPB = 6                          # blocks per output chunk
NCH = (NB + CPB - 1) // CPB      # 8 chunks
NEG = -1e9

_NCS = {}
REPS = int(os.environ.get("BASS_KERNEL_REPS", "1"))
DBG = os.environ.get("BASS_KERNEL_DBG", "0") == "1"
DBG_J = 7          # debug block (chunk 1)


def _qw(j):
    return min(QB, S - QB * j)


def _build(reps=None):
    reps = REPS if reps is None else reps
    import concourse.bacc as bacc
    import concourse.mybir as mybir
    from concourse.tile import TileContext

    F32 = mybir.dt.float32
    F16 = mybir.dt.float16
    BF16 = mybir.dt.bfloat16
    EXP = mybir.ActivationFunctionType.Exp
    MULT = mybir.AluOpType.mult

    nc = bacc.Bacc(None, target_bir_lowering=False)

    xq_d = nc.dram_tensor("xqT", [D, S], F16, kind="ExternalInput")
    xk_d = nc.dram_tensor("xkT", [D, S], F16, kind="ExternalInput")
    xv_d = nc.dram_tensor("xvT", [D, S], F16, kind="ExternalInput")
    wq_d = nc.dram_tensor("wqT", [D, D], F16, kind="ExternalInput")
    wk_d = nc.dram_tensor("wkT", [D, D], F16, kind="ExternalInput")
    wv_d = nc.dram_tensor("wvT", [D, D], F16, kind="ExternalInput")
    wo_d = nc.dram_tensor("woT", [D, D], F16, kind="ExternalInput")
    bqc_d = nc.dram_tensor("bqc", [128, 4], F32, kind="ExternalInput")
    bkc_d = nc.dram_tensor("bkc", [128, 4], F32, kind="ExternalInput")
    boc_d = nc.dram_tensor("boc", [128, 4], F32, kind="ExternalInput")
    pen_d = nc.dram_tensor("pen", [128, NB], F32, kind="ExternalInput")
    band_d = nc.dram_tensor("band8", [128, H * QB], BF16, kind="ExternalInput")
    id_d = nc.dram_tensor("ident", [128, 128], F16, kind="ExternalInput")
    one_d = nc.dram_tensor("ones", [128, 1], BF16, kind="ExternalInput")
    qz_d = nc.dram_tensor("qzero", [64, 4, S], F16, kind="ExternalInput")
    kz_d = nc.dram_tensor("kzero", [128, 4, 112], F16, kind="ExternalInput")
    out_d = nc.dram_tensor("out", [D, S], F16, kind="ExternalOutput")
    if DBG:
        dbg_qZ = nc.dram_tensor("dbg_qZ", [128, H, S], F16, kind="ExternalOutput")
        dbg_kT = nc.dram_tensor("dbg_kT", [128, 4, HALF + S + 112], F16,
                                kind="ExternalOutput")
        dbg_vt = nc.dram_tensor("dbg_vt", [128, H, HD], BF16, kind="ExternalOutput")
        dbg_pT = nc.dram_tensor("dbg_pT", [128, H, QB], BF16, kind="ExternalOutput")
        dbg_att = nc.dram_tensor("dbg_att", [QB, D], F16, kind="ExternalOutput")
        dbg_atT = nc.dram_tensor("dbg_atT", [128, 4, CPB * QB], F16,
                                 kind="ExternalOutput")
        dbg_st = nc.dram_tensor("dbg_st", [128, H, QB], F32, kind="ExternalOutput")

    def r4(t):  # [512, N] dram -> [128, 4, N] view
        return t[:, :].rearrange("(c p) n -> p c n", p=128)

    XVW = (CPB - 1) * QB + 128 + 2 * HALF    # 608: staged xv window per chunk

    with TileContext(nc) as tc:
        with tc.tile_pool(name="const", bufs=1) as cp, \
             tc.tile_pool(name="stq", bufs=2) as stq, \
             tc.tile_pool(name="stk", bufs=2) as stk, \
             tc.tile_pool(name="stv", bufs=2) as stv, \
             tc.tile_pool(name="vtp", bufs=3) as vtp, \
             tc.tile_pool(name="ptp", bufs=3) as ptp, \
             tc.tile_pool(name="attp", bufs=3) as attp, \
             tc.tile_pool(name="rcp", bufs=2) as rcp, \
             tc.tile_pool(name="atp", bufs=2) as atp, \
             tc.tile_pool(name="otp", bufs=2) as otp, \
             tc.tile_pool(name="projp", bufs=3, space="PSUM") as projp, \
             tc.tile_pool(name="stpa", bufs=1, space="PSUM") as stpa, \
             tc.tile_pool(name="stpb", bufs=1, space="PSUM") as stpb, \
             tc.tile_pool(name="m2p", bufs=1, space="PSUM") as m2p, \
             tc.tile_pool(name="denp", bufs=1, space="PSUM") as denp, \
             tc.tile_pool(name="trp", bufs=1, space="PSUM") as trp:

            # ---- constants / persistent state (outside the rep loop) ----
            wq_sb = cp.tile([128, 4, D], F16, name="wq_sb")
            wk_sb = cp.tile([128, 4, D], F16, name="wk_sb")
            wv_sb = cp.tile([128, 4, D], F16, name="wv_sb")
            wo_sb = cp.tile([128, 4, D], F16, name="wo_sb")
            nc.sync.dma_start(wq_sb[:], r4(wq_d))
            nc.sync.dma_start(wk_sb[:], r4(wk_d))
            nc.sync.dma_start(wv_sb[:], r4(wv_d))
            nc.sync.dma_start(wo_sb[:], r4(wo_d))
            bqc = cp.tile([128, 4], F32, name="bqc_sb")
            bkc = cp.tile([128, 4], F32, name="bkc_sb")
            boc = cp.tile([128, 4], F32, name="boc_sb")
            pen = cp.tile([128, NB], F32, name="pen_sb")
            band = cp.tile([128, H, QB], BF16, name="band_sb")
            iden = cp.tile([128, 128], F16, name="id_sb")
            ones = cp.tile([128, 1], BF16, name="ones_sb")
            nc.sync.dma_start(bqc[:], bqc_d[:, :])
            nc.sync.dma_start(bkc[:], bkc_d[:, :])
            nc.sync.dma_start(boc[:], boc_d[:, :])
            nc.sync.dma_start(pen[:], pen_d[:, :])
            nc.sync.dma_start(band[:], band_d[:, :].rearrange("p (h q) -> p h q", q=QB))
            nc.sync.dma_start(iden[:], id_d[:, :])
            nc.sync.dma_start(ones[:], one_d[:, :])

            # persistent q/k: qZ per head with the co-projected head's
            # partition half zeroed (base-64 matmul operands fault at runtime,
            # so MM1 is pair-packed over full-128 contractions instead).
            qZ = cp.tile([128, H, S], F16, name="qZ")
            nc.sync.dma_start(qZ[0:64, 1:H:2, :], qz_d[:, :, :])
            nc.sync.dma_start(qZ[64:128, 0:H:2, :], qz_d[:, :, :])
            kT = cp.tile([128, 4, HALF + S + 112], F16, name="kT")
            nc.sync.dma_start(kT[:, :, 0:HALF], kz_d[:, :, 0:HALF])
            nc.sync.dma_start(kT[:, :, HALF + S:], kz_d[:, :, :])

            loop_cm = (tc.For_i(0, reps, 1) if reps > 1
                       else contextlib.nullcontext())
            with loop_cm:
                xv_tiles = {}

                def stage_xv(ci):
                    j0 = ci * CPB
                    j1 = min(NB, j0 + CPB) - 1
                    lo = QB * j0 - HALF
                    hi = QB * j1 + 128 - HALF
                    wid = hi - lo
                    t = stv.tile([128, 4, XVW], F16, tag="xv", name="xv")
                    dlo, dhi = max(0, lo), min(S, hi)
                    nc.sync.dma_start(t[:, :, dlo - lo:dhi - lo],
                                      r4(xv_d)[:, :, dlo:dhi])
                    if dlo > lo:
                        nc.vector.memset(t[:, :, 0:dlo - lo], 0.0)
                    if dhi < hi:
                        nc.vector.memset(t[:, :, dhi - lo:wid], 0.0)
                    xv_tiles[ci] = t

                stage_xv(0)

                # ---- phase 1: q/k projections over full S ----
                for ti in range(S // 512):
                    sl = slice(512 * ti, 512 * ti + 512)
                    xq_st = stq.tile([128, 4, 512], F16, tag="xq", name="xq")
                    nc.sync.dma_start(xq_st[:], r4(xq_d)[:, :, sl])
                    xk_st = stk.tile([128, 4, 512], F16, tag="xk", name="xk")
                    nc.sync.dma_start(xk_st[:], r4(xk_d)[:, :, sl])
                    for dc in range(4):
                        ps = projp.tile([128, 512], F32, tag="proj", name="pps")
                        for k in range(4):
                            nc.tensor.matmul(ps[:],
                                             wq_sb[:, k, 128 * dc:128 * dc + 128],
                                             xq_st[:, k, :],
                                             start=(k == 0), stop=(k == 3))
                        nc.vector.tensor_scalar_add(
                            qZ[0:64, 2 * dc, sl], ps[0:64, :], bqc[0:64, dc:dc + 1])
                        nc.scalar.add(
                            qZ[64:128, 2 * dc + 1, sl], ps[64:128, :],
                            bqc[64:128, dc:dc + 1])
                        ps2 = projp.tile([128, 512], F32, tag="proj", name="pks")
                        for k in range(4):
                            nc.tensor.matmul(ps2[:],
                                             wk_sb[:, k, 128 * dc:128 * dc + 128],
                                             xk_st[:, k, :],
                                             start=(k == 0), stop=(k == 3))
                        keng = nc.vector if dc % 2 == 0 else nc.scalar
                        if dc % 2 == 0:
                            nc.vector.tensor_scalar_add(
                                kT[:, dc, HALF + 512 * ti:HALF + 512 * ti + 512],
                                ps2[:], bkc[:, dc:dc + 1])
                        else:
                            nc.scalar.add(
                                kT[:, dc, HALF + 512 * ti:HALF + 512 * ti + 512],
                                ps2[:], bkc[:, dc:dc + 1])

                # ---- phase 2: blocks, 2-deep software pipeline ----
                vts, pts, m2s, dens, rcs, atts = {}, {}, {}, {}, {}, {}
                atT_tiles = {}

                def front_pe(j):
                    ci = j // CPB
                    if j % CPB == 0 and ci + 1 < NCH:
                        stage_xv(ci + 1)
                    xv_st = xv_tiles[ci]
                    vloc = QB * (j - ci * CPB)
                    ps = projp.tile([128, 512], F32, tag="proj", name="vps")
                    for k in range(4):
                        nc.tensor.matmul(ps[:],
                                         xv_st[:, k, vloc:vloc + 128],
                                         wv_sb[:, k, :],
                                         start=(k == 0), stop=(k == 3))
                    vt = vtp.tile([128, H, HD], BF16, tag="vt", name="vt")
                    nc.scalar.copy(
                        vt[:, :, :], ps[:].rearrange("p (h d) -> p h d", d=64))
                    vts[j] = vt
                    if DBG and j == DBG_J:
                        nc.sync.dma_start(dbg_vt[:, :, :], vt[:, :, :])
                    qw = _qw(j)
                    stA = stpa.tile([128, 4, QB], F32, tag="stA", name="stA")
                    stB = stpb.tile([128, 4, QB], F32, tag="stB", name="stB")
                    for p in range(4):
                        st = stA if p < 2 else stB
                        po = 2 * (p % 2)
                        if qw == QB:
                            nc.tensor.matmul(
                                st[:, po:po + 2, :],
                                kT[:, p, QB * j:QB * j + 128],
                                qZ[:, 2 * p:2 * p + 2, QB * j:QB * j + qw],
                                start=True, stop=True)
                        else:
                            for hh in range(2):
                                nc.tensor.matmul(
                                    st[:, po + hh, :qw],
                                    kT[:, p, QB * j:QB * j + 128],
                                    qZ[:, 2 * p + hh, QB * j:QB * j + qw],
                                    start=True, stop=True)
                    return stA, stB

                def emit_exp(j, sts_):
                    stA, stB = sts_
                    qw = _qw(j)
                    if DBG and j == DBG_J:
                        sto = attp.tile([128, H, QB], F32, tag="sto", name="sto")
                        nc.vector.tensor_copy(sto[:, 0:4, :qw], stA[:, :, :qw])
                        nc.vector.tensor_copy(sto[:, 4:8, :qw], stB[:, :, :qw])
                        nc.sync.dma_start(dbg_st[:, :, :qw], sto[:, :, :qw])
                    pT = ptp.tile([128, H, QB], BF16, tag="pT", name="pT")
                    nc.scalar.activation(pT[:, 0:4, :qw], stA[:, :, :qw], EXP,
                                         bias=pen[:, j:j + 1], scale=1.0)
                    nc.scalar.activation(pT[:, 4:8, :qw], stB[:, :, :qw], EXP,
                                         bias=pen[:, j:j + 1], scale=1.0)
                    pts[j] = pT

                def emit_band(j):
                    qw = _qw(j)
                    pT = pts[j]
                    nc.gpsimd.tensor_tensor(out=pT[:, :, :qw], in0=pT[:, :, :qw],
                                            in1=band[:, :, :qw], op=MULT)
                    if DBG and j == DBG_J:
                        nc.sync.dma_start(dbg_pT[:, :, :qw], pT[:, :, :qw])

                def mid_pe(j):
                    qw = _qw(j)
                    pT, vt = pts[j], vts[j]
                    m2 = m2p.tile([QB, H, HD], F32, tag="m2", name="m2")
                    den = denp.tile([QB, H], F32, tag="den", name="den")
                    for h in range(H):
                        nc.tensor.matmul(m2[:qw, h, :], pT[:, h, :qw],
                                         vt[:, h, :], start=True, stop=True)
                        nc.tensor.matmul(den[:qw, h:h + 1], pT[:, h, :qw],
                                         ones[:, :], start=True, stop=True)
                    m2s[j], dens[j] = m2, den

                def mid_dve(j):
                    qw = _qw(j)
                    m2, den = m2s[j], dens[j]
                    rc = rcp.tile([QB, H], F32, tag="rc", name="rc")
                    nc.vector.reciprocal(rc[:qw, :], den[:qw, :])
                    att = attp.tile([QB, D], F16, tag="att", name="att")
                    attv = att.rearrange("q (h d) -> q h d", d=64)
                    nc.vector.tensor_tensor(
                        out=attv[:qw, :, :],
                        in0=m2[:qw, :, :],
                        in1=rc[:qw, :].unsqueeze(2).to_broadcast((qw, H, 64)),
                        op=MULT)
                    atts[j] = att
                    if DBG and j == DBG_J:
                        nc.sync.dma_start(dbg_att[:qw, :], att[:qw, :])

                oT_tiles = {}

                def op_group(ci, t, dc, nw):
                    if ci not in oT_tiles:
                        oT_tiles[ci] = otp.tile([128, 4, CPB * QB], F16,
                                                tag="oT", name="oT")
                    oT = oT_tiles[ci]
                    atT = atT_tiles[ci]
                    op = projp.tile([128, 512], F32, tag="proj", name="ops")
                    for k in range(4):
                        nc.tensor.matmul(
                            op[:, :nw],
                            wo_sb[:, k, 128 * dc:128 * dc + 128],
                            atT[:, k, nw * t:nw * t + nw],
                            start=(k == 0), stop=(k == 3))
                    if dc % 2 == 0:
                        nc.scalar.add(oT[:, dc, nw * t:nw * t + nw],
                                      op[:, :nw], boc[:, dc:dc + 1])
                    else:
                        nc.vector.tensor_scalar_add(
                            oT[:, dc, nw * t:nw * t + nw],
                            op[:, :nw], boc[:, dc:dc + 1])

                def op_dma(ci):
                    j0 = ci * CPB
                    qwid = min(S, QB * (j0 + CPB)) - QB * j0
                    nc.sync.dma_start(r4(out_d)[:, :, QB * j0:QB * j0 + qwid],
                                      oT_tiles[ci][:, :, :qwid])

                # outproj schedule: group (t, dc) of chunk ci is ready once
                # atT cols [288t, 288t+288) are written, i.e. after
                # back(j0+3t+2) which runs at iteration j0+3t+4. Spread 2
                # groups per iteration; DMA after the last group.
                op_sched = {}
                for ci in range(NCH):
                    j0 = ci * CPB
                    qwid = min(S, QB * (j0 + CPB)) - QB * j0
                    if qwid == CPB * QB:
                        nw = 288
                        for t in range(2):
                            base = j0 + 3 * t + 4
                            op_sched.setdefault(base, []).append((ci, t, 0, nw))
                            op_sched.setdefault(base, []).append((ci, t, 1, nw))
                            op_sched.setdefault(base + 1, []).append((ci, t, 2, nw))
                            op_sched.setdefault(base + 1, []).append((ci, t, 3, nw))
                        op_sched.setdefault(j0 + 8, []).append((ci, "dma", 0, 0))
                    else:
                        base = j0 + (qwid + QB - 1) // QB + 2
                        for dc in range(4):
                            op_sched.setdefault(base + dc // 2, []).append(
                                (ci, 0, dc, qwid))
                        op_sched.setdefault(base + 2, []).append((ci, "dma", 0, 0))

                def back(j):
                    ci = j // CPB
                    qw = _qw(j)
                    if j % CPB == 0:
                        atT_tiles[ci] = atp.tile([128, 4, CPB * QB], F16,
                                                 tag="atT", name="atT")
                    atT = atT_tiles[ci]
                    qloc = QB * (j - ci * CPB)
                    att = atts[j]
                    tr = trp.tile([128, 4, QB], F16, tag="tr", name="tr")
                    for i in range(4):
                        nc.tensor.transpose(tr[:, i, :qw],
                                            att[:qw, 128 * i:128 * i + 128],
                                            iden[:qw, :qw])
                    nc.vector.tensor_copy(atT[:, :, qloc:qloc + qw], tr[:, :, :qw])

                sts = {}
                for i in range(NB + 6):
                    if i < NB:
                        sts[i] = front_pe(i)
                    if 0 <= i - 1 < NB:
                        mid_pe(i - 1)
                    if i < NB:
                        emit_exp(i, sts[i])
                    if 0 <= i - 1 < NB:
                        mid_dve(i - 1)
                    if i < NB:
                        emit_band(i)
                    if 0 <= i - 2 < NB:
                        back(i - 2)
                    for (ci, t, dc, nw) in op_sched.get(i, []):
                        if t == "dma":
                            if DBG and ci == DBG_J // CPB:
                                nc.sync.dma_start(dbg_atT[:, :, :],
                                                  atT_tiles[ci][:, :, :])
                            op_dma(ci)
                        else:
                            op_group(ci, t, dc, nw)
                if DBG:
                    nc.sync.dma_start(dbg_qZ[:, :, :], qZ[:, :, :])
                    nc.sync.dma_start(dbg_kT[:, :, :], kT[:, :, :])

    nc.finalize()
    return nc


def _host_consts():
    rr = np.arange(128)[:, None]
    qq = np.arange(QB)[None, :]
    band = (((rr - qq) >= 0) & ((rr - qq) <= 32)).astype(np.float32)
    band8 = np.tile(band, (1, H))
    ident = np.eye(128, dtype=np.float32)
    return band8, ident


def _get_nc(reps=None):
    reps = REPS if reps is None else reps
    if reps not in _NCS:
        _NCS[reps] = _build(reps)
    return _NCS[reps]


def _prep_inmaps(query, key, value, mask, Wq, bq, Wk, bk, Wv, bv, Wo, bo):
    query = np.asarray(query, np.float32)
    key = np.asarray(key, np.float32)
    value = np.asarray(value, np.float32)
    mask = np.asarray(mask)
    Wq, bq = np.asarray(Wq, np.float32), np.asarray(bq, np.float32)
    Wk, bk = np.asarray(Wk, np.float32), np.asarray(bk, np.float32)
    Wv, bv = np.asarray(Wv, np.float32), np.asarray(bv, np.float32)
    Wo, bo = np.asarray(Wo, np.float32), np.asarray(bo, np.float32)

    band8, ident = _host_consts()
    boc = (Wo @ bv + bo).reshape(4, 128).T.astype(np.float32)
    jj = np.arange(NB)[None, :]
    rr = np.arange(128)[:, None]
    pos = QB * jj - HALF + rr                      # [128, NB]
    valid = (pos >= 0) & (pos < S)
    posc = np.clip(pos, 0, S - 1)

    import ml_dtypes
    bf16 = ml_dtypes.bfloat16
    f16 = np.float16
    common = {
        "wqT": np.ascontiguousarray(Wq.T).astype(f16),
        "wkT": np.ascontiguousarray(Wk.T).astype(f16),
        "wvT": np.ascontiguousarray(Wv.T).astype(f16),
        "woT": np.ascontiguousarray(Wo.T).astype(f16),
        "bqc": np.ascontiguousarray(bq.reshape(4, 128).T),
        "bkc": np.ascontiguousarray(bk.reshape(4, 128).T),
        "boc": np.ascontiguousarray(boc),
        "band8": band8.astype(bf16),
        "ident": ident.astype(f16),
        "ones": np.ones((128, 1), bf16),
        "qzero": np.zeros((64, 4, S), f16),
        "kzero": np.zeros((128, 4, 112), f16),
    }
    from concurrent.futures import ThreadPoolExecutor

    def _one(b):
        pen = np.where(valid & ~mask[b][posc], 0.0, NEG).astype(np.float32)
        return dict(
            common,
            xqT=np.ascontiguousarray(query[b].T).astype(f16),
            xkT=np.ascontiguousarray(key[b].T).astype(f16),
            xvT=np.ascontiguousarray(value[b].T).astype(f16),
            pen=pen,
        )

    with ThreadPoolExecutor(max_workers=8) as ex:
        in_maps = list(ex.map(_one, range(B)))
    return in_maps


def kernel(**inputs):
    from concourse.bass_utils import run_bass_kernel_spmd
    in_maps = _prep_inmaps(**inputs)
    res = run_bass_kernel_spmd(_get_nc(), in_maps, core_ids=list(range(8)))
    return np.stack(
        [res.results[c]["out"].T.astype(np.float32) for c in range(B)], axis=0)
